# revision 7
# baseline (speedup 1.0000x reference)
"""Trainium2 Bass kernel for nn_NetworkEncoderRoPE_85968065396755.

Dual-branch geometry-aware cross-attention encoder block.
Strategy: pure data-parallel over batch B=256 across 8 NeuronCores
(32 batches/core). All weights replicated and SBUF-resident in bf16;
LayerNorm gains/biases and attention scales folded into the weight
matrices host-side, so the device only does: normalize, matmuls,
softmax, gelu/sigmoid, residuals.

Self-contained: only imports installed packages (concourse/jax/numpy).
"""

import os
import numpy as np
import ml_dtypes

import concourse.bass as bass  # noqa: F401
import concourse.tile as tile
from concourse import bacc, mybir
from concourse.bass_utils import run_bass_kernel_spmd
from concourse.masks import make_identity

F32 = mybir.dt.float32
BF16 = mybir.dt.bfloat16
AF = mybir.ActivationFunctionType
ALU = mybir.AluOpType
BF = ml_dtypes.bfloat16

# Problem dims (hardcoded per contract)
B, L, D = 256, 120, 512
H, HD, DS, DF = 8, 64, 128, 2048
GEO_SCALE = 0.5
EPS = 1e-5
NCORES = 8
BPC = B // NCORES  # 32 batches per core

_BUILD_CACHE = {}
LAST_RESULTS = None  # test.py introspection


# ----------------------------------------------------------------------------
# host-side weight folding / packing
# ----------------------------------------------------------------------------

def _pack_w(w):
    """[Din, Dout] -> [128, Din//128, Dout] so that [:, kc, :] is the
    contraction chunk kc in natural [din, dout] layout."""
    din, dout = w.shape
    return np.ascontiguousarray(
        w.reshape(din // 128, 128, dout).transpose(1, 0, 2).astype(BF)
    )


def _pack_b(b):
    """[Dout] -> [128, Dout//128] per-partition bias chunks (fp32)."""
    n = b.shape[0]
    return np.ascontiguousarray(b.reshape(n // 128, 128).T.astype(np.float32))


def _prep_weights(params):
    p = {k: np.asarray(v, np.float32) for k, v in params.items()}
    out = {}
    s_hd = HD ** -0.5
    s_ds = DS ** -0.5
    for i in range(2):
        g_q, b_q = p["ln_q_g"][i], p["ln_q_b"][i]
        g_kv, b_kv = p["ln_kv_g"][i], p["ln_kv_b"][i]

        wq = (g_q[:, None] * p["Wq"][i]) * s_hd
        bq = (b_q @ p["Wq"][i] + p["bq"][i]) * s_hd
        wk = g_kv[:, None] * p["Wk"][i]
        bk = b_kv @ p["Wk"][i] + p["bk"][i]
        wv = g_kv[:, None] * p["Wv"][i]
        bv = b_kv @ p["Wv"][i] + p["bv"][i]
        wsq = (g_q[:, None] * p["Wsq"][i]) * s_ds
        bsq = (b_q @ p["Wsq"][i] + p["bsq"][i]) * s_ds
        wsk = g_kv[:, None] * p["Wsk"][i]
        bsk = b_kv @ p["Wsk"][i] + p["bsk"][i]
        wo = p["Wo"][i]
        # v-bias folded through Wo (softmax rows sum to 1)
        bo = p["bo"][i] + bv @ wo

        gwx = p["gate_W"][i][:D]
        gwh = p["gate_W"][i][D:]
        gb = p["gate_b"][i]

        g_f, b_f = p["ffn_ln_g"][i], p["ffn_ln_b"][i]
        w1 = g_f[:, None] * p["ffn_W1"][i]
        b1 = b_f @ p["ffn_W1"][i] + p["ffn_b1"][i]
        w2 = p["ffn_W2"][i]
        b2 = p["ffn_b2"][i]

        out[f"wq{i}"] = _pack_w(wq)
        out[f"wk{i}"] = _pack_w(wk)
        out[f"wv{i}"] = _pack_w(wv)
        out[f"wo{i}"] = _pack_w(wo)
        out[f"wsq{i}"] = _pack_w(wsq)
        out[f"wsk{i}"] = _pack_w(wsk)
        out[f"gwx{i}"] = _pack_w(gwx)
        out[f"gwh{i}"] = _pack_w(gwh)
        out[f"w1{i}"] = _pack_w(w1)
        out[f"w2{i}"] = _pack_w(w2)
        out[f"bq{i}"] = _pack_b(bq)
        out[f"bk{i}"] = _pack_b(bk)
        out[f"bsq{i}"] = _pack_b(bsq)
        out[f"bsk{i}"] = _pack_b(bsk)
        out[f"b1{i}"] = _pack_b(b1)
        out[f"bo{i}"] = np.ascontiguousarray(bo[None, :].astype(BF))
        out[f"gb{i}"] = np.ascontiguousarray(gb[None, :].astype(BF))
        out[f"b2{i}"] = np.ascontiguousarray(b2[None, :].astype(BF))
    return out


# ----------------------------------------------------------------------------
# device program
# ----------------------------------------------------------------------------

def _emit(tc, dram):
    nc = tc.nc
    import contextlib

    with contextlib.ExitStack() as ctx:
        consts = ctx.enter_context(tc.tile_pool(name="consts", bufs=1))
        wpool = ctx.enter_context(tc.tile_pool(name="wpool", bufs=1))
        work = ctx.enter_context(tc.tile_pool(name="work", bufs=2))
        xin = ctx.enter_context(tc.tile_pool(name="xin", bufs=3))
        stat = ctx.enter_context(tc.tile_pool(name="stat", bufs=6))
        psum = ctx.enter_context(tc.tile_pool(name="psum", bufs=1, space="PSUM"))

        # --- constants ---
        ident = consts.tile([128, 128], BF16, name="ident", tag="ident")
        make_identity(nc, ident)
        ones_col = consts.tile([128, 1], BF16, name="ones_col", tag="ones_col")
        nc.vector.memset(ones_col, 1.0)
        ones_row = consts.tile([1, 128], BF16, name="ones_row", tag="ones_row")
        nc.vector.memset(ones_row, 1.0)
        eps_t = consts.tile([128, 1], F32, name="eps_t", tag="eps_t")
        nc.vector.memset(eps_t, EPS)

        # --- resident weights ---
        wt = {}
        for nm, d in dram["w"].items():
            t = wpool.tile(list(d.shape), d.dtype, name=f"t_{nm}", tag=f"t_{nm}")
            nc.sync.dma_start(out=t, in_=d)
            wt[nm] = t

        xp_d, xv_d = dram["x_pose"], dram["x_vis"]
        out_d = [dram["out_pose"], dram["out_vis"]]

        def ln_normalize(x, tagp):
            """fp32 [L,D] -> bf16 normalized z [L,D] (no gain/bias)."""
            st6 = stat.tile([L, 6], F32, name=f"st6_{tagp}", tag="st6")
            nc.vector.bn_stats(st6, x)
            mv = stat.tile([L, 2], F32, name=f"mv_{tagp}", tag="mv")
            nc.vector.bn_aggr(mv, st6)
            rstd = stat.tile([L, 1], F32, name=f"rstd_{tagp}", tag="rstd")
            nc.scalar.activation(rstd, mv[:, 1:2], AF.Sqrt, bias=eps_t[:L])
            nc.vector.reciprocal(rstd, rstd)
            nmr = stat.tile([L, 1], F32, name=f"nmr_{tagp}", tag="nmr")
            nc.vector.tensor_scalar(nmr, mv[:, 0:1], rstd, -1.0, ALU.mult, ALU.mult)
            z = work.tile([L, D], BF16, name=f"z_{tagp}", tag="z", bufs=3)
            nc.scalar.activation(z, x, AF.Identity, bias=nmr, scale=rstd)
            return z

        def transpose_512(z, name):
            """bf16 [L, 512] -> bf16 [128, 4, L] (chunked transpose)."""
            zT = work.tile([128, 4, L], BF16, name=name, tag=name)
            for c in range(4):
                pt = psum.tile([128, L], BF16, name=f"pt_{name}_{c}", tag="small",
                               bufs=5)
                nc.tensor.transpose(pt, z[:, c * 128:(c + 1) * 128],
                                    ident[:L, :L])
                nc.vector.tensor_copy(zT[:, c, :], pt)
            return zT

        for b in range(BPC):
            # ---------- stage A: load + shared LN + transposes ----------
            xp = xin.tile([L, D], F32, name=f"xp{b}", tag="xp")
            nc.sync.dma_start(out=xp, in_=xp_d[b])
            xv = xin.tile([L, D], F32, name=f"xv{b}", tag="xv")
            nc.sync.dma_start(out=xv, in_=xv_d[b])

            zp = ln_normalize(xp, f"p{b}")
            zpT = transpose_512(zp, "zpT")
            zv = ln_normalize(xv, f"v{b}")
            zvT = transpose_512(zv, "zvT")

            for i in range(2):
                if i == 0:
                    xq, zqT, zkT = xp, zpT, zvT
                else:
                    xq, zqT, zkT = xv, zvT, zpT

                # ---------- B: projections ----------
                qT = work.tile([128, 4, L], BF16, name=f"qT{b}_{i}", tag="qT")
                kT = work.tile([128, 4, L], BF16, name=f"kT{b}_{i}", tag="kT")
                for dst, w_t, b_t, src in (
                    (qT, wt[f"wq{i}"], wt[f"bq{i}"], zqT),
                    (kT, wt[f"wk{i}"], wt[f"bk{i}"], zkT),
                ):
                    for d4 in range(4):
                        pm = psum.tile([128, L], F32, name=f"pm{b}_{i}_{d4}",
                                       tag="small", bufs=5)
                        for kc in range(4):
                            nc.tensor.matmul(
                                pm, lhsT=w_t[:, kc, d4 * 128:(d4 + 1) * 128],
                                rhs=src[:, kc, :],
                                start=(kc == 0), stop=(kc == 3))
                        nc.scalar.activation(dst[:, d4, :], pm, AF.Identity,
                                             bias=b_t[:, d4:d4 + 1])

                qsT = work.tile([128, L], BF16, name=f"qsT{b}_{i}", tag="qsT")
                ksT = work.tile([128, L], BF16, name=f"ksT{b}_{i}", tag="ksT")
                for dst, w_t, b_t, src in (
                    (qsT, wt[f"wsq{i}"], wt[f"bsq{i}"], zqT),
                    (ksT, wt[f"wsk{i}"], wt[f"bsk{i}"], zkT),
                ):
                    pm = psum.tile([128, L], F32, name=f"pms{b}_{i}", tag="small",
                                   bufs=5)
                    for kc in range(4):
                        nc.tensor.matmul(pm, lhsT=w_t[:, kc, :], rhs=src[:, kc, :],
                                         start=(kc == 0), stop=(kc == 3))
                    nc.scalar.activation(dst, pm, AF.Identity, bias=b_t[:, 0:1])

                # V (standard layout [tok, 512]); bias folded into bo
                pv = psum.tile([L, D], F32, name=f"pv{b}_{i}", tag="big", bufs=3)
                for kc in range(4):
                    nc.tensor.matmul(pv, lhsT=zkT[:, kc, :],
                                     rhs=wt[f"wv{i}"][:, kc, :],
                                     start=(kc == 0), stop=(kc == 3))
                v_sb = work.tile([L, D], BF16, name=f"v{b}_{i}", tag="v")
                nc.vector.tensor_copy(v_sb, pv)

                # sim[j, kk] = sum_d ksim[j,d] qsim[kk,d]
                psim = psum.tile([L, L], F32, name=f"psim{b}_{i}", tag="small",
                                 bufs=5)
                nc.tensor.matmul(psim, lhsT=ksT, rhs=qsT, start=True, stop=True)
                sim_sb = work.tile([L, L], BF16, name=f"sim{b}_{i}", tag="sim")
                nc.vector.tensor_copy(sim_sb, psim)

                # ---------- C: attention per head ----------
                aoT = work.tile([128, 4, L], BF16, name=f"aoT{b}_{i}", tag="aoT")
                for h in range(8):
                    c, po = h // 2, (h % 2) * 64
                    ps = psum.tile([L, L], F32, name=f"ps{b}_{i}_{h}", tag="small",
                                   bufs=5)
                    # scores[q,k] (scaled q) + geo bias via accumulation
                    nc.tensor.matmul(ps, lhsT=qT[po:po + 64, c, :],
                                     rhs=kT[po:po + 64, c, :],
                                     start=True, stop=False)
                    nc.tensor.matmul(ps, lhsT=wt["atT"][:L, :L], rhs=sim_sb,
                                     start=False, stop=True)
                    # softmax over k (free dim); no max-sub (scores are small)
                    exp_sb = work.tile([L, L], BF16, name=f"exp{b}_{i}_{h}",
                                       tag="exp", bufs=3)
                    sums = stat.tile([L, 1], F32, name=f"sums{b}_{i}_{h}",
                                     tag="sums")
                    nc.scalar.activation(exp_sb, ps, AF.Exp, accum_out=sums)
                    recip = stat.tile([L, 1], F32, name=f"recip{b}_{i}_{h}",
                                      tag="recip")
                    nc.vector.reciprocal(recip, sums)
                    attn = work.tile([L, L], BF16, name=f"attn{b}_{i}_{h}",
                                     tag="attn", bufs=3)
                    nc.vector.tensor_scalar(attn, exp_sb, recip, None, ALU.mult)
                    # transpose attn -> [k, q]
                    pat = psum.tile([L, L], BF16, name=f"pat{b}_{i}_{h}",
                                    tag="small", bufs=5)
                    nc.tensor.transpose(pat, attn, ident[:L, :L])
                    attnT = work.tile([L, L], BF16, name=f"attnT{b}_{i}_{h}",
                                      tag="attnT", bufs=3)
                    nc.scalar.copy(attnT, pat)
                    # out_avT[d, q]
                    pav = psum.tile([HD, L], F32, name=f"pav{b}_{i}_{h}",
                                    tag="small", bufs=5)
                    nc.tensor.matmul(pav, lhsT=v_sb[:, h * 64:(h + 1) * 64],
                                     rhs=attnT, start=True, stop=True)
                    nc.vector.tensor_copy(aoT[po:po + 64, c, :], pav)

                # ---------- D: Wo, gate, residual ----------
                ph = psum.tile([L, D], F32, name=f"ph{b}_{i}", tag="big", bufs=3)
                for kc in range(4):
                    nc.tensor.matmul(ph, lhsT=aoT[:, kc, :],
                                     rhs=wt[f"wo{i}"][:, kc, :],
                                     start=(kc == 0), stop=False)
                nc.tensor.matmul(ph, lhsT=ones_row[:, :L], rhs=wt[f"bo{i}"],
                                 start=False, stop=True)
                h_sb = work.tile([L, D], BF16, name=f"h{b}_{i}", tag="h")
                nc.vector.tensor_copy(h_sb, ph)

                hT = transpose_512(h_sb, "hT")
                xq_bf = work.tile([L, D], BF16, name=f"xqbf{b}_{i}", tag="xqbf")
                nc.vector.tensor_copy(xq_bf, xq)
                xqT = transpose_512(xq_bf, "xqT")

                pg = psum.tile([L, D], F32, name=f"pg{b}_{i}", tag="big", bufs=3)
                for kc in range(4):
                    nc.tensor.matmul(pg, lhsT=xqT[:, kc, :],
                                     rhs=wt[f"gwx{i}"][:, kc, :],
                                     start=(kc == 0), stop=False)
                for kc in range(4):
                    nc.tensor.matmul(pg, lhsT=hT[:, kc, :],
                                     rhs=wt[f"gwh{i}"][:, kc, :],
                                     start=False, stop=False)
                nc.tensor.matmul(pg, lhsT=ones_row[:, :L], rhs=wt[f"gb{i}"],
                                 start=False, stop=True)
                g_sb = work.tile([L, D], BF16, name=f"g{b}_{i}", tag="g")
                nc.scalar.activation(g_sb, pg, AF.Sigmoid)
                # g := g * h  (in place)
                nc.vector.tensor_mul(g_sb, g_sb, h_sb)
                x_out = work.tile([L, D], F32, name=f"xo{b}_{i}", tag="xo",
                                  bufs=3)
                nc.vector.tensor_add(x_out, xq, g_sb)

                # ---------- E: FFN ----------
                z2 = ln_normalize(x_out, f"f{b}_{i}")
                z2T = transpose_512(z2, "z2T")
                h1T = work.tile([128, 16, L], BF16, name=f"h1T{b}_{i}", tag="h1T")
                for d16 in range(16):
                    pf = psum.tile([128, L], F32, name=f"pf{b}_{i}_{d16}",
                                   tag="small", bufs=5)
                    for kc in range(4):
                        nc.tensor.matmul(
                            pf, lhsT=wt[f"w1{i}"][:, kc, d16 * 128:(d16 + 1) * 128],
                            rhs=z2T[:, kc, :],
                            start=(kc == 0), stop=(kc == 3))
                    nc.scalar.activation(h1T[:, d16, :], pf, AF.Gelu,
                                         bias=wt[f"b1{i}"][:, d16:d16 + 1])

                pw2 = psum.tile([L, D], F32, name=f"pw2{b}_{i}", tag="big",
                                bufs=3)
                for kc in range(16):
                    nc.tensor.matmul(pw2, lhsT=h1T[:, kc, :],
                                     rhs=wt[f"w2{i}"][:, kc, :],
                                     start=(kc == 0), stop=False)
                nc.tensor.matmul(pw2, lhsT=ones_row[:, :L], rhs=wt[f"b2{i}"],
                                 start=False, stop=True)
                x_fin = work.tile([L, D], F32, name=f"xf{b}_{i}", tag="xf",
                                  bufs=3)
                nc.vector.tensor_add(x_fin, x_out, pw2)
                nc.sync.dma_start(out=out_d[i][b], in_=x_fin)


def _build(weight_specs):
    key = f"v1_{BPC}"
    if key in _BUILD_CACHE:
        return _BUILD_CACHE[key]

    nc = bacc.Bacc("TRN2", target_bir_lowering=False, debug=False)
    dram = {"w": {}}
    dram["x_pose"] = nc.dram_tensor("x_pose", [BPC, L, D], F32,
                                    kind="ExternalInput").ap()
    dram["x_vis"] = nc.dram_tensor("x_vis", [BPC, L, D], F32,
                                   kind="ExternalInput").ap()
    for nm, arr in weight_specs.items():
        dt = BF16 if arr.dtype == BF else F32
        dram["w"][nm] = nc.dram_tensor(nm, list(arr.shape), dt,
                                       kind="ExternalInput").ap()
    dram["out_pose"] = nc.dram_tensor("out_pose", [BPC, L, D], F32,
                                      kind="ExternalOutput").ap()
    dram["out_vis"] = nc.dram_tensor("out_vis", [BPC, L, D], F32,
                                     kind="ExternalOutput").ap()

    with tile.TileContext(nc) as tc:
        _emit(tc, dram)
    nc.compile()

    _BUILD_CACHE[key] = (nc, None)
    return _BUILD_CACHE[key]


def kernel(x_pose, x_vis, time_adjacency, params):
    global LAST_RESULTS
    x_pose = np.asarray(x_pose, np.float32)
    x_vis = np.asarray(x_vis, np.float32)
    A = np.asarray(time_adjacency, np.float32)

    w = _prep_weights(params)
    # pre-scaled, pre-transposed adjacency: atT[j, q] = GEO_SCALE * A[q, j]
    w["atT"] = np.ascontiguousarray((GEO_SCALE * A).T.astype(BF))

    nc, _ = _build(w)

    core_ids = list(range(NCORES))
    in_maps = []
    for c in range(NCORES):
        m = dict(w)
        m["x_pose"] = np.ascontiguousarray(x_pose[c * BPC:(c + 1) * BPC])
        m["x_vis"] = np.ascontiguousarray(x_vis[c * BPC:(c + 1) * BPC])
        in_maps.append(m)

    trace = bool(int(os.environ.get("BASS_KERNEL_TRACE", "0")))
    res = run_bass_kernel_spmd(nc, in_maps, core_ids, trace=trace)
    LAST_RESULTS = res

    out_pose = np.concatenate([np.asarray(r["out_pose"]) for r in res.results], 0)
    out_vis = np.concatenate([np.asarray(r["out_vis"]) for r in res.results], 0)
    return out_pose.astype(np.float32), out_vis.astype(np.float32)
